# revision 14
# baseline (speedup 1.0000x reference)
"""Log2Quantizer Trainium2 kernel (raw Bass, no Tile).

Math: the reference's sort/std/rank machinery is dead code (bit_token is
unconditionally overwritten with n_bits), so the computation reduces to:
    delta[b,t] = max over (h,c) of x[b,h,t,c]
    out = delta * 2^(round(log2(max(x/delta, 1e-8))))
Bit-trick route (no transcendentals), exact on fp32 engines:
    q  = x * (1/(delta*sqrt2));  p2 = bitcast_f32(bits(q) & 0x7F800000)
    out = p2 * (2*delta)
Sharding: data-parallel over batch dim b (8 rows -> 8 cores), no comms.

Perf structure (DMA roofline ~70us busy: 12.6MB in + 12.6MB out per core at
~360GB/s; ~5.7us NEFF preamble + ~2us postamble on top):
  - Every DMA costs P*H = 1536 descriptors regardless of chunk size, and
    descriptor overhead is partly exposed on the engines (DMA_15 straggles
    ~1.2x). So the middle of the tensor is LOADED as 1024-token chunks
    (2KB runs, half the load DMAs), while STORES stay at 512-token
    granularity (two half-stores per big chunk, issued as each half's M2
    completes) - a single 3MB store per big chunk was tried and starves
    the store stream (6us DMA gaps).
  - Small chunks at START (pull the store stream earlier - the first store
    gates on c0's load+compute chain) and END (short final dependency
    chains). Last two chunks: all-DVE M2, stores issued by the idle sync
    engine straight off dve_sem (skips the ACT queue's issue lag).
  - One SBUF slot per chunk in two big [P, 96KB] arenas (xt: input then
    AND output; wt: q then final output) -> no recycling; all loads queue
    on the SP HWDGE ring at t=0, ACT-ring stores overlap them.
  - Compute split so neither engine paces below the DMA rate:
      DVE (0.96GHz): reduce_max (1x), per-token scalars, AND (2x), and
                     half the M2 (p2*2delta) slices (2x port mode)
      ACT (1.2GHz):  all M1 (q = x*invs, Copy w/ per-partition scale AP),
                     the other half of M2, and most store issues
  - Explicit fences (wait own counting sem) between dependent DVE ops; a
    dma_start does NOT wait for the issuing engine's in-flight compute, so
    ACT stores self-fence on act_sem. Block(no_gpsimd_drain=True) was
    tried and is ~14% slower (gpsimd firmware contends for the DVE/GpSimd
    shared SBUF port, stalling every DVE 2x-port op).
"""

from contextlib import ExitStack

import numpy as np

import concourse.bass as bass
import concourse.mybir as mybir
from concourse.bass_utils import run_bass_kernel_spmd

B, H, T, C = 8, 12, 4096, 64
N_CORES = 8
P = 128

# chunk sizes (tokens), sum = T; 1024-token middles are stored in 2 halves
TCS = [128, 128, 256, 1024, 1024, 1024, 256, 128, 128]
assert sum(TCS) == T and all(tc % P == 0 for tc in TCS)
NCH = len(TCS)
TTS = [tc // P for tc in TCS]              # tokens per partition per chunk
OFFS = np.cumsum([0] + TTS).tolist()       # per-partition token offsets
N_SYNC_STORES = 2                          # last chunks: sync-issued stores

SQRT2 = 1.4142135623730951
EXP_MASK = 0x7F800000
TPP = T // P                               # 32 tokens per partition total

_nc_cache = {}


def _plan():
    """Per-chunk M2-slice assignment and store units.

    Returns per chunk: (dve_slices, store_units) where each store unit is
    (name, act_m2_slices, dve_m2_slices, half_index, n_halves).
    Big (tt=8) chunks: DVE does slices {0,1,4,5}, ACT {2,3,6,7}; two
    half-stores (q 0-3 and q 4-7). Small chunks: DVE does tt//2 slices
    (ALL tt for the last N_SYNC_STORES chunks), one whole-chunk store.
    """
    plan = []
    for ci in range(NCH):
        tt = TTS[ci]
        if tt == 8:
            plan.append(
                (
                    [0, 1, 4, 5],
                    [("A", [2, 3], [0, 1], 0, 2), ("B", [6, 7], [4, 5], 1, 2)],
                )
            )
        else:
            k = tt if ci >= NCH - N_SYNC_STORES else tt // 2
            plan.append(
                ((list(range(k)), [("W", list(range(k, tt)), list(range(k)), 0, 1)]))
            )
    return plan


PLAN = _plan()


def _build_nc():
    if "nc" in _nc_cache:
        return _nc_cache["nc"]
    f32 = mybir.dt.float32
    i32 = mybir.dt.int32
    OP = mybir.AluOpType
    Copy = mybir.ActivationFunctionType.Copy

    nc = bass.Bass()
    x_in = nc.declare_dram_parameter("x", [H, T, C], f32, isOutput=False)
    y_out = nc.declare_dram_parameter("y", [H, T, C], f32, isOutput=True)

    def load_ap(ci):
        t0 = OFFS[ci] * P
        return x_in[:, t0 : t0 + TCS[ci], :].rearrange(
            "h (p q) c -> p h (q c)", p=P
        )

    def store_ap(ci, half, nh):
        t0 = OFFS[ci] * P
        r = y_out[:, t0 : t0 + TCS[ci], :]
        if nh == 1:
            return r.rearrange("h (p q) c -> p h (q c)", p=P)
        # half of a big chunk: tokens p*tt + half*tt/2 + [0, tt/2)
        return r.rearrange("h (p s q) c -> p s h (q c)", p=P, s=2)[
            :, half : half + 1
        ]

    def sb_half(arena, ci, half, nh):
        # SBUF view matching store_ap(ci, half, nh)
        off = OFFS[ci] * H * C
        sz = TTS[ci] * H * C
        r = arena[:, off : off + sz]
        if nh == 1:
            return r
        return r.rearrange("p (h s q c) -> p s h (q c)", h=H, s=2, c=C)[
            :, half : half + 1
        ]

    # ---- dry-run of the ACT op schedule (python counters -> wait targets)
    m1_done = [0] * NCH
    m2act_done = {}  # (ci, unit_name) -> act_sem count after that unit's M2act
    actn = 0
    for ci in range(NCH):
        actn += TTS[ci]
        m1_done[ci] = actn
        if ci >= 1 and ci - 1 < NCH - N_SYNC_STORES:
            for name, act_s, _dve_s, _h, _nh in PLAN[ci - 1][1]:
                actn += len(act_s)
                m2act_done[(ci - 1, name)] = actn

    with ExitStack() as ctx:
        xt = ctx.enter_context(nc.sbuf_tensor("xt", [P, TPP * H * C], f32))
        wt = ctx.enter_context(nc.sbuf_tensor("wt", [P, TPP * H * C], f32))
        xt_i = xt[:].bitcast(i32)
        wt_i = wt[:].bitcast(i32)
        delta = ctx.enter_context(nc.sbuf_tensor("delta", [P, max(TTS)], f32))
        ds = ctx.enter_context(nc.sbuf_tensor("ds", [P, max(TTS)], f32))
        inv = ctx.enter_context(nc.sbuf_tensor("inv", [P, TPP], f32))
        d2 = ctx.enter_context(nc.sbuf_tensor("d2", [P, TPP], f32))

        load_sem = ctx.enter_context(nc.semaphore("load_sem"))
        store_sem = ctx.enter_context(nc.semaphore("store_sem"))
        act_sem = ctx.enter_context(nc.semaphore("act_sem"))
        dve_sem = ctx.enter_context(nc.semaphore("dve_sem"))

        dve_n = 0
        recip_done = [0] * NCH
        and_done = [0] * NCH
        m2dve_done = {}       # (ci, unit_name) -> dve_sem count
        n_stores = 0          # number of store DMAs emitted

        def csl(ci):
            tt = TTS[ci]
            off = OFFS[ci] * H * C
            sz = tt * H * C
            return (
                tt,
                OFFS[ci],
                xt[:, off : off + sz],
                xt_i[:, off : off + sz],
                wt_i[:, off : off + sz],
                xt[:, off : off + sz].rearrange("p (h q c) -> p h q c", h=H, c=C),
                wt[:, off : off + sz].rearrange("p (h q c) -> p h q c", h=H, c=C),
            )

        block = ctx.enter_context(nc.Block())

        @block.sync
        def _(sync):
            for ci in range(NCH):
                sync.dma_start(out=csl(ci)[2], in_=load_ap(ci)).then_inc(
                    load_sem, 16
                )

        @block.vector
        def _(vector):
            def emit_front(ci):
                nonlocal dve_n
                tt, toff, xs, xsi, wsi, xs4, ws4 = csl(ci)
                vector.wait_ge(load_sem, 16 * (ci + 1))
                if ci >= 1:
                    # WAR on delta/ds: prior chunk's front must be complete
                    vector.wait_ge(dve_sem, recip_done[ci - 1])
                vector.reduce_max(
                    out=delta[:, 0:tt],
                    in_=xs4.transpose([0, 2, 1, 3]),
                    axis=mybir.AxisListType.XY,
                ).then_inc(dve_sem, 1)
                dve_n += 1
                vector.wait_ge(dve_sem, dve_n)  # fence: ds/d2 read delta
                vector.tensor_scalar_mul(
                    ds[:, 0:tt], delta[:, 0:tt], SQRT2
                ).then_inc(dve_sem, 1)
                dve_n += 1
                vector.tensor_scalar_mul(
                    d2[:, toff : toff + tt], delta[:, 0:tt], 2.0
                ).then_inc(dve_sem, 1)
                dve_n += 1
                vector.wait_ge(dve_sem, dve_n)  # fence: recip reads ds
                vector.reciprocal(
                    inv[:, toff : toff + tt], ds[:, 0:tt]
                ).then_inc(dve_sem, 1)
                dve_n += 1
                recip_done[ci] = dve_n

            def emit_back(ci):
                nonlocal dve_n
                tt, toff, xs, xsi, wsi, xs4, ws4 = csl(ci)
                vector.wait_ge(act_sem, m1_done[ci])
                vector.tensor_scalar(
                    out=xsi,
                    in0=wsi,
                    scalar1=EXP_MASK,
                    scalar2=None,
                    op0=OP.bitwise_and,
                ).then_inc(dve_sem, 1)
                dve_n += 1
                and_done[ci] = dve_n
                vector.wait_ge(dve_sem, dve_n)  # fence: M2 reads AND output
                for name, _act_s, dve_s, _h, _nh in PLAN[ci][1]:
                    for s in dve_s:
                        vector.tensor_scalar_mul(
                            ws4[:, :, s, :],
                            xs4[:, :, s, :],
                            d2[:, toff + s : toff + s + 1],
                        ).then_inc(dve_sem, 1)
                        dve_n += 1
                    m2dve_done[(ci, name)] = dve_n

            emit_front(0)
            for ci in range(1, NCH):
                emit_front(ci)
                emit_back(ci - 1)
            emit_back(NCH - 1)

        @block.scalar
        def _(scalar):
            nonlocal_actn = [0]

            def emit_m1(ci):
                tt, toff, xs, xsi, wsi, xs4, ws4 = csl(ci)
                scalar.wait_ge(dve_sem, recip_done[ci])
                for s in range(tt):
                    scalar.activation(
                        ws4[:, :, s, :],
                        xs4[:, :, s, :],
                        Copy,
                        bias=0.0,
                        scale=inv[:, toff + s : toff + s + 1],
                    ).then_inc(act_sem, 1)
                nonlocal_actn[0] += tt
                assert nonlocal_actn[0] == m1_done[ci]

            def emit_units(ci):
                nonlocal n_stores
                tt, toff, xs, xsi, wsi, xs4, ws4 = csl(ci)
                scalar.wait_ge(dve_sem, and_done[ci])
                for name, act_s, _dve_s, half, nh in PLAN[ci][1]:
                    for s in act_s:
                        scalar.activation(
                            ws4[:, :, s, :],
                            xs4[:, :, s, :],
                            Copy,
                            bias=0.0,
                            scale=d2[:, toff + s : toff + s + 1],
                        ).then_inc(act_sem, 1)
                    nonlocal_actn[0] += len(act_s)
                    assert nonlocal_actn[0] == m2act_done[(ci, name)]
                    # data fences: DVE's M2 part + this engine's own M2act
                    scalar.wait_ge(dve_sem, m2dve_done[(ci, name)])
                    if act_s:
                        scalar.wait_ge(act_sem, m2act_done[(ci, name)])
                    scalar.dma_start(
                        out=store_ap(ci, half, nh), in_=sb_half(wt, ci, half, nh)
                    ).then_inc(store_sem, 16)
                    n_stores += 1

            emit_m1(0)
            for ci in range(1, NCH):
                emit_m1(ci)
                if ci - 1 < NCH - N_SYNC_STORES:
                    emit_units(ci - 1)

        # second sync section, emitted AFTER the vector block so the
        # m2dve_done targets hold their final values
        @block.sync
        def _(sync):
            nonlocal n_stores
            for ci in range(NCH - N_SYNC_STORES, NCH):
                sync.wait_ge(dve_sem, m2dve_done[(ci, "W")])
                sync.dma_start(
                    out=store_ap(ci, 0, 1), in_=sb_half(wt, ci, 0, 1)
                ).then_inc(store_sem, 16)
                n_stores += 1
            sync.wait_ge(store_sem, 16 * n_stores)  # final store fence

    _nc_cache["nc"] = nc
    return nc


def kernel(x: np.ndarray) -> np.ndarray:
    assert x.shape == (B, H, T, C) and x.dtype == np.float32
    nc = _build_nc()
    in_maps = [{"x": np.ascontiguousarray(x[i])} for i in range(N_CORES)]
    res = run_bass_kernel_spmd(nc, in_maps, list(range(N_CORES)))
    out = np.stack([res.results[i]["y"] for i in range(N_CORES)], axis=0)
    return out


# revision 15
# speedup vs baseline: 1.1181x; 1.1181x over previous
"""Log2Quantizer Trainium2 kernel (raw Bass, no Tile).

Math: the reference's sort/std/rank machinery is dead code (bit_token is
unconditionally overwritten with n_bits), so the computation reduces to:
    delta[b,t] = max over (h,c) of x[b,h,t,c]
    out = delta * 2^(round(log2(max(x/delta, 1e-8))))
i.e. snap x/delta to the nearest power of two in log space, rescale by delta.

Division-route bit-trick (no transcendentals), exact on fp32 engines:
    q  = x * (1/(delta*sqrt2))               (reciprocal is IEEE 1/x on trn2)
    p2 = bitcast_f32(bits(q) & 0x7F800000)   # 2^floor(log2 q) = 2^(k-1)
    out = p2 * (2*delta)                     # fp32 mult by 2^k, exact
round(log2(x/delta)) = floor(log2(x/(delta*sqrt2))) + 1, so flooring q to its
exponent implements the rounding; x==0 gives q=0 -> p2=+0.0 -> out~0.

Sharding: data-parallel over batch dim b (8 rows -> 8 cores), no comms.

Perf structure (target: DMA roofline ~70us = 12.6MB in + 12.6MB out at the
~358GB/s HBM-per-core limit; measured DMA busy-rate is ~363GB/s):
  - Tokens split into chunks, TAPERED at both ends (128/256-token chunks
    first/last, 512 in the middle) so the first reduce starts ~5us earlier
    and the last store's dependency chain is short.
  - One SBUF slot per chunk in two big [P, 24KB] arenas (xt: input then
    AND output; wt: q then final output) -> no recycling, all loads queue
    on the SP HWDGE ring at t=0, stores go out on the ACT ring.
  - Compute split so neither engine paces below the DMA rate:
      DVE (0.96GHz): reduce_max (1x), tiny per-token scalars, AND (2x),
                     and HALF the M2 (p2*2delta) slices (2x port mode)
      ACT (1.2GHz):  all M1 (q = x*invs, Copy w/ per-partition scale AP),
                     the other half of M2, and store issue
    ~6.8us/chunk on DVE vs ~5.8us on ACT vs ~8.3us/chunk of DMA.
  - Explicit fences (wait on own counting sem) between dependent DVE ops;
    cross-engine deps via python-counted absolute semaphore targets.
"""

from contextlib import ExitStack

import numpy as np

import concourse.bass as bass
import concourse.mybir as mybir
from concourse.bass_utils import run_bass_kernel_spmd

B, H, T, C = 8, 12, 4096, 64
N_CORES = 8
P = 128

# chunk sizes (tokens), sum = T. Small chunks at the START pull the whole
# store stream earlier (first store issue gates on c0's load+compute chain);
# small chunks at the END shorten the last stores' dependency chains. 512 in
# the middle: 1024-token chunks were tried and starve the store stream
# (lumpy 12us compute per chunk -> 6us DMA gaps); 512 keeps the flow smooth.
TCS = [128, 128, 256, 512, 512, 512, 512, 512, 512, 256, 128, 128]
assert sum(TCS) == T and all(tc % P == 0 for tc in TCS)
NCH = len(TCS)
TTS = [tc // P for tc in TCS]              # tokens per partition per chunk
KS = [tt // 2 for tt in TTS]               # M2 slices done on DVE (rest ACT)
KS[-2] = TTS[-2]                           # last two chunks: all M2 on DVE,
KS[-1] = TTS[-1]                           # stores issued by the sync engine
N_SYNC_STORES = 2
OFFS = np.cumsum([0] + TTS).tolist()       # per-partition token offsets

SQRT2 = 1.4142135623730951
EXP_MASK = 0x7F800000
TPP = T // P                               # 32 tokens per partition total

_nc_cache = {}


def _build_nc():
    if "nc" in _nc_cache:
        return _nc_cache["nc"]
    f32 = mybir.dt.float32
    i32 = mybir.dt.int32
    OP = mybir.AluOpType
    Copy = mybir.ActivationFunctionType.Copy

    nc = bass.Bass()
    x_in = nc.declare_dram_parameter("x", [H, T, C], f32, isOutput=False)
    y_out = nc.declare_dram_parameter("y", [H, T, C], f32, isOutput=True)

    def dram_ap(t, ci):
        t0 = OFFS[ci] * P
        return t[:, t0 : t0 + TCS[ci], :].rearrange(
            "h (p q) c -> p h (q c)", p=P
        )

    # ACT-side op schedule (python dry-run): per chunk, M1 x tt, then (for
    # the previous chunk) M2act x (tt - tt//2); needed by DVE's AND wait and
    # by the store's self-fence (a dma_start does NOT wait for the issuing
    # engine's in-flight compute, so the store must wait act_sem).
    m1_done = [0] * NCH
    m2act_done = [0] * NCH
    actn = 0
    for ci in range(NCH):
        actn += TTS[ci]
        m1_done[ci] = actn
        if ci >= 1:
            actn += TTS[ci - 1] - KS[ci - 1]
            m2act_done[ci - 1] = actn
    actn += TTS[NCH - 1] - KS[NCH - 1]
    m2act_done[NCH - 1] = actn

    with ExitStack() as ctx:
        xt = ctx.enter_context(nc.sbuf_tensor("xt", [P, TPP * H * C], f32))
        wt = ctx.enter_context(nc.sbuf_tensor("wt", [P, TPP * H * C], f32))
        xt_i = xt[:].bitcast(i32)
        wt_i = wt[:].bitcast(i32)
        delta = ctx.enter_context(nc.sbuf_tensor("delta", [P, max(TTS)], f32))
        ds = ctx.enter_context(nc.sbuf_tensor("ds", [P, max(TTS)], f32))
        inv = ctx.enter_context(nc.sbuf_tensor("inv", [P, TPP], f32))
        d2 = ctx.enter_context(nc.sbuf_tensor("d2", [P, TPP], f32))

        load_sem = ctx.enter_context(nc.semaphore("load_sem"))
        store_sem = ctx.enter_context(nc.semaphore("store_sem"))
        act_sem = ctx.enter_context(nc.semaphore("act_sem"))
        dve_sem = ctx.enter_context(nc.semaphore("dve_sem"))

        # python-side counters -> absolute wait targets
        dve_n = 0
        recip_done = [0] * NCH
        and_done = [0] * NCH
        m2dve_done = [0] * NCH

        def csl(ci):
            # chunk slice in the big arenas + 4D views + per-token scalars
            tt = TTS[ci]
            off = OFFS[ci] * H * C
            sz = tt * H * C
            return (
                tt,
                OFFS[ci],
                xt[:, off : off + sz],
                wt[:, off : off + sz],
                xt_i[:, off : off + sz],
                wt_i[:, off : off + sz],
                xt[:, off : off + sz].rearrange("p (h q c) -> p h q c", h=H, c=C),
                wt[:, off : off + sz].rearrange("p (h q c) -> p h q c", h=H, c=C),
            )

        # NOTE: Block(no_gpsimd_drain=True) was tried and is ~14% SLOWER:
        # without the gpsimd drain, gpsimd firmware activity contends for the
        # DVE/GpSimd shared SBUF port and stalls every DVE 2x-port op.
        block = ctx.enter_context(nc.Block())

        @block.sync
        def _(sync):
            for ci in range(NCH):
                sync.dma_start(out=csl(ci)[2], in_=dram_ap(x_in, ci)).then_inc(
                    load_sem, 16
                )

        @block.vector
        def _(vector):
            def emit_front(ci):
                nonlocal dve_n
                tt, toff, xs, ws, xsi, wsi, xs4, ws4 = csl(ci)
                vector.wait_ge(load_sem, 16 * (ci + 1))
                if ci >= 1:
                    # WAR on delta/ds: prior chunk's front must be complete
                    vector.wait_ge(dve_sem, recip_done[ci - 1])
                vector.reduce_max(
                    out=delta[:, 0:tt],
                    in_=xs4.transpose([0, 2, 1, 3]),
                    axis=mybir.AxisListType.XY,
                ).then_inc(dve_sem, 1)
                dve_n += 1
                vector.wait_ge(dve_sem, dve_n)  # fence: ds/d2 read delta
                vector.tensor_scalar_mul(
                    ds[:, 0:tt], delta[:, 0:tt], SQRT2
                ).then_inc(dve_sem, 1)
                dve_n += 1
                vector.tensor_scalar_mul(
                    d2[:, toff : toff + tt], delta[:, 0:tt], 2.0
                ).then_inc(dve_sem, 1)
                dve_n += 1
                vector.wait_ge(dve_sem, dve_n)  # fence: recip reads ds
                vector.reciprocal(
                    inv[:, toff : toff + tt], ds[:, 0:tt]
                ).then_inc(dve_sem, 1)
                dve_n += 1
                recip_done[ci] = dve_n

            def emit_back(ci):
                nonlocal dve_n
                tt, toff, xs, ws, xsi, wsi, xs4, ws4 = csl(ci)
                vector.wait_ge(act_sem, m1_done[ci])
                vector.tensor_scalar(
                    out=xsi,
                    in0=wsi,
                    scalar1=EXP_MASK,
                    scalar2=None,
                    op0=OP.bitwise_and,
                ).then_inc(dve_sem, 1)
                dve_n += 1
                and_done[ci] = dve_n
                k = KS[ci]
                if k:
                    vector.wait_ge(dve_sem, dve_n)  # fence: M2 reads AND out
                    for s in range(k):
                        vector.tensor_scalar_mul(
                            ws4[:, :, s, :],
                            xs4[:, :, s, :],
                            d2[:, toff + s : toff + s + 1],
                        ).then_inc(dve_sem, 1)
                        dve_n += 1
                m2dve_done[ci] = dve_n

            emit_front(0)
            for ci in range(1, NCH):
                emit_front(ci)
                emit_back(ci - 1)
            emit_back(NCH - 1)

        @block.scalar
        def _(scalar):
            def emit_m1(ci):
                tt, toff, xs, ws, xsi, wsi, xs4, ws4 = csl(ci)
                scalar.wait_ge(dve_sem, recip_done[ci])
                for s in range(tt):
                    scalar.activation(
                        ws4[:, :, s, :],
                        xs4[:, :, s, :],
                        Copy,
                        bias=0.0,
                        scale=inv[:, toff + s : toff + s + 1],
                    ).then_inc(act_sem, 1)

            def emit_m2act_store(ci):
                tt, toff, xs, ws, xsi, wsi, xs4, ws4 = csl(ci)
                k = KS[ci]
                scalar.wait_ge(dve_sem, and_done[ci])
                for s in range(k, tt):
                    scalar.activation(
                        ws4[:, :, s, :],
                        xs4[:, :, s, :],
                        Copy,
                        bias=0.0,
                        scale=d2[:, toff + s : toff + s + 1],
                    ).then_inc(act_sem, 1)
                if k:
                    scalar.wait_ge(dve_sem, m2dve_done[ci])
                # self-fence: the M2act slices above must have COMPLETED
                # (not merely issued) before the store DMA reads ws
                scalar.wait_ge(act_sem, m2act_done[ci])
                scalar.dma_start(out=dram_ap(y_out, ci), in_=ws).then_inc(
                    store_sem, 16
                )

            emit_m1(0)
            for ci in range(1, NCH):
                emit_m1(ci)
                if ci - 1 < NCH - N_SYNC_STORES:
                    emit_m2act_store(ci - 1)

        # second sync section, emitted AFTER the vector block so the
        # m2dve_done[] targets hold their final values (the first sync
        # section is emitted before vector and must not read them)
        @block.sync
        def _(sync):
            for ci in range(NCH - N_SYNC_STORES, NCH):
                # all-DVE M2 for these chunks -> store straight off dve_sem,
                # skipping the ACT queue's issue lag; SP ring walker is idle
                # by now so descriptor gen starts immediately
                sync.wait_ge(dve_sem, m2dve_done[ci])
                sync.dma_start(out=dram_ap(y_out, ci), in_=csl(ci)[3]).then_inc(
                    store_sem, 16
                )
            sync.wait_ge(store_sem, 16 * NCH)  # final store fence

    _nc_cache["nc"] = nc
    return nc


def kernel(x: np.ndarray) -> np.ndarray:
    assert x.shape == (B, H, T, C) and x.dtype == np.float32
    nc = _build_nc()
    in_maps = [{"x": np.ascontiguousarray(x[i])} for i in range(N_CORES)]
    res = run_bass_kernel_spmd(nc, in_maps, list(range(N_CORES)))
    out = np.stack([res.results[i]["y"] for i in range(N_CORES)], axis=0)
    return out
